# revision 39
# baseline (speedup 1.0000x reference)
"""Trainium2 Bass kernel for nn_BinaryLoss (BCE triangle-mesh loss).

Structure
---------
Host (integer combinatorics on the tiny index tensors only; no FP math on
logits): sorted-triangle key table -> unique keys; undirected GT edge set;
per-vertex unique-triangle counts; candidate-triple membership gt_mask
[N,256] via searchsorted; manifold row mask w [N]; edge mask gm [N,16].
Two exact identities drive the device plan:
  * gt_labels_masked == gt_mask (a GT triangle always contributes its own
    (e0,e1) edge to full_mat, so the dense adjacency lookup is redundant),
  * sum_m [sp(x) - x*mask] needs only softplus sums plus the sum of x over
    masked positions (<= 8 per row here, gathered to a narrow [rows,L]).
Only manifold rows (w==1, ~800 of 16384) contribute to the main loss, so
just those rows' logits ship to the device.

Device (all logit FP math, 8 cores data-parallel, per core):
  * gsel = compacted gm==1 groups of 16 logits, exp()'d on ScalarE
    (monotonic, ranks unchanged) then the DVE Max8 instruction gives the
    exact descending top-8 per group: rank1=exp(pos), rank2=exp(neg).
    sp(-pos)=Ln(1+1/exp(pos)) via DVE reciprocal + Ln, sp(neg)=Ln(1+exp(neg))
    -- no exp needed after the single activation-table switch.
  * selected rows: softplus via Exp then Ln(1+e) on ScalarE (transposed
    layout), per-partition sums on DVE.
  * ScalarE/DVE instruction orders are pinned with explicit dep edges to
    keep one exp->ln table transition and a stall-free DVE tail.
  * per-core raw partial sums [128,6] DMA out; the host applies inv_denom /
    inv_cnt and does the cross-core/partition scalar all-reduce.
Pad rows/groups use +-30 logits so their softplus terms are ~1e-13.
"""
import os
import numpy as np

N_CORES = 8
B_PAD = 30.0  # pad-group magnitude: softplus(-30) ~ 9e-14


# ---------------------------------------------------------------- host prep
def _host_prep(pred_logits, points, knn_indices, gt_triangles):
    N, K = knn_indices.shape
    M = (K - 1) * (K - 1)
    num_pts = points.shape[0]
    P = num_pts + 1

    tri = np.sort(np.asarray(gt_triangles, dtype=np.int64), axis=1)
    keys = tri[:, 0] * (P * P) + tri[:, 1] * P + tri[:, 2]
    uk = np.unique(keys)

    ut0, ut1, ut2 = uk // (P * P), (uk // P) % P, uk % P
    counts = np.zeros(P, np.float64)
    np.add.at(counts, ut0, 1.0)
    np.add.at(counts, ut1, (ut1 != ut0).astype(np.float64))
    np.add.at(counts, ut2, (ut2 != ut1).astype(np.float64))
    all_N_gt = counts[np.asarray(knn_indices[:, 0], dtype=np.int64)]

    e_u = np.concatenate([np.minimum(tri[:, 0], tri[:, 1]),
                          np.minimum(tri[:, 1], tri[:, 2]),
                          np.minimum(tri[:, 0], tri[:, 2])])
    e_v = np.concatenate([np.maximum(tri[:, 0], tri[:, 1]),
                          np.maximum(tri[:, 1], tri[:, 2]),
                          np.maximum(tri[:, 0], tri[:, 2])])
    ekeys = np.unique(e_u * P + e_v)

    c = np.asarray(knn_indices[:, 0], dtype=np.int64)[:, None]
    a = np.asarray(knn_indices[:, 1:], dtype=np.int64)
    q = np.minimum(c, a) * P + np.maximum(c, a)
    pos = np.clip(np.searchsorted(ekeys, q.ravel()), 0, len(ekeys) - 1)
    gm = (ekeys[pos] == q.ravel()).reshape(N, K - 1)

    e0 = np.repeat(a, K - 1, axis=1)
    e1 = np.tile(a, (1, K - 1))
    v0 = np.broadcast_to(c, e0.shape)
    cand = np.stack([v0, e0, e1], axis=-1)
    cand.sort(axis=-1)
    ck = cand[..., 0] * (P * P) + cand[..., 1] * P + cand[..., 2]
    cpos = np.clip(np.searchsorted(uk, ck.ravel()), 0, len(uk) - 1)
    gt_mask = (uk[cpos] == ck.ravel()).reshape(N, M)

    all_N_pred = gt_mask.sum(1).astype(np.float64)
    manifold = (all_N_gt * 2.0) == all_N_pred
    w = manifold.astype(np.float32)

    inv_denom = np.float32(1.0 / max(float(w.sum(dtype=np.float64)) * M, 1.0))
    inv_cnt = np.float32(1.0 / max(float(gm.sum(dtype=np.float64)), 1.0))
    return gt_mask, gm, w, inv_denom, inv_cnt


def _make_shards(x, gt_mask, gm, w, inv_denom, inv_cnt):
    """Build per-core input dicts. x is [N,256] f32."""
    N, M = x.shape
    parts = 128

    # masked-x values padded to L per row (L chosen from data)
    mask_per_row = gt_mask.sum(1)
    L = max(8, int(mask_per_row.max()))
    L = int(2 ** np.ceil(np.log2(L)))
    rr, cc = np.nonzero(gt_mask)
    xm = np.zeros((N, L), np.float32)
    row_starts = np.zeros(N + 1, np.int64)
    np.add.at(row_starts, rr + 1, 1)
    row_starts = np.cumsum(row_starts)
    ranks = np.arange(len(rr)) - row_starts[rr]
    xm[rr, ranks] = x[rr, cc]

    # only manifold rows (w==1) contribute to the main BCE: select them
    sel = np.nonzero(w)[0]
    W = len(sel)
    cap_pc = max(parts, int(np.ceil(W / (N_CORES * parts))) * parts)
    CAP = cap_pc * N_CORES
    xs = np.full((CAP, M), -B_PAD, np.float32)   # pad rows: softplus ~ 1e-13
    xs[:W] = x[sel]
    xms = np.zeros((CAP, L), np.float32)
    xms[:W] = xm[sel]

    # compacted gm groups, padded; distributed evenly over cores
    gn, gi = np.nonzero(gm)               # group ids (row, i)
    total = len(gn)
    per_core = int(np.ceil(total / N_CORES))
    g_chunks = max(1, int(np.ceil(per_core / parts)))  # free-dim group chunks
    cap = g_chunks * parts                       # groups per core
    pl3 = x.reshape(N, 16, 16)

    pad_group = np.full(16, -B_PAD, np.float32)
    pad_group[0] = B_PAD
    pad_group[1] = B_PAD

    in_maps = []
    for core in range(N_CORES):
        s0, s1 = core * cap_pc, (core + 1) * cap_pc
        xc = np.ascontiguousarray(xs[s0:s1].T)          # [256, cap_pc] f32
        kk = cap_pc // parts
        xmc = np.ascontiguousarray(xms[s0:s1]).reshape(parts, kk * L)

        lo, hi = core * per_core, min((core + 1) * per_core, total)
        gsel = np.broadcast_to(pad_group, (cap, 16)).copy()
        if hi > lo:
            gsel[: hi - lo] = pl3[gn[lo:hi], gi[lo:hi], :]
        gsel = np.ascontiguousarray(
            gsel.reshape(g_chunks, parts, 16).transpose(1, 0, 2)
        ).reshape(parts, g_chunks * 16)

        in_maps.append({"x": xc, "xm": xmc, "gsel": gsel})
    return in_maps, L, g_chunks, cap_pc


# ---------------------------------------------------------------- bass build
def _build_bass(L, g_chunks, cap_pc):
    from contextlib import ExitStack

    import concourse.bacc as bacc
    import concourse.mybir as mybir
    import concourse.tile as tile

    f32 = mybir.dt.float32
    bf16 = mybir.dt.bfloat16
    AFT = mybir.ActivationFunctionType
    ALU = mybir.AluOpType
    AX = mybir.AxisListType

    parts, rpp = 128, 16
    G = g_chunks
    S = cap_pc          # selected rows per core
    KK = S // parts     # xm row-chunks per partition

    nc = bacc.Bacc(
        "TRN2", target_bir_lowering=False, debug=False,
        enable_asserts=False, num_devices=N_CORES,
    )
    x_d = nc.dram_tensor("x", [2 * parts, S], f32, kind="ExternalInput").ap()
    xm_d = nc.dram_tensor("xm", [parts, KK * L], f32, kind="ExternalInput").ap()
    g_d = nc.dram_tensor("gsel", [parts, G * 16], f32, kind="ExternalInput").ap()
    out_d = nc.dram_tensor("out", [128, 6], f32, kind="ExternalOutput").ap()

    with tile.TileContext(nc) as tc, ExitStack() as ctx:
        from concourse.tile import add_dep_helper

        def chain(lst):
            for a, b in zip(lst, lst[1:]):
                add_dep_helper(b.ins, a.ins, sync=True, reason="engine order")

        pool = ctx.enter_context(tc.tile_pool(name="main", bufs=1))

        acts = []  # explicit ScalarE program order (avoids table-load thrash)
        dves = []  # pinned DVE order for the post-Max8 tail

        # hoist the exp-table load: dummy activation with no DMA deps
        dumt = pool.tile([1, 8], f32)
        nc.vector.memset(dumt[:], 0.0)
        dumo = pool.tile([1, 8], f32)
        acts.append(nc.scalar.activation(dumo[:], dumt[:], AFT.Exp))

        # --- DMAs: gsel first (feeds the critical Max8 chain) ---
        gt = pool.tile([parts, G * 16], f32)
        NGC = 4
        gch = G * 16 // NGC
        for i in range(NGC):
            nc.sync.dma_start(gt[:, i * gch:(i + 1) * gch],
                              g_d[:, i * gch:(i + 1) * gch])
        halves = []
        for h in range(2):
            xth = pool.tile([parts, S], f32, name=f"xt{h}", tag=f"xt{h}")
            nc.gpsimd.dma_start(xth[:], x_d[h * parts:(h + 1) * parts, :])
            halves.append(xth)
        xmt = pool.tile([parts, KK * L], f32)
        nc.gpsimd.dma_start(xmt[:], xm_d[:])

        # --- exp over gsel (monotonic: Max8 ranks unchanged) interleaved
        #     with the selected-row exps so the ln table switch lands early
        ge = pool.tile([parts, G * 16], f32)
        ets, sps = [], []
        for h in range(2):
            ets.append(pool.tile([parts, S], f32, name=f"e{h}", tag=f"e{h}"))
            sps.append(pool.tile([parts, S], f32, name=f"sp{h}", tag=f"sp{h}"))
        for i in range(NGC):
            acts.append(nc.scalar.activation(ge[:, i * gch:(i + 1) * gch],
                                             gt[:, i * gch:(i + 1) * gch],
                                             AFT.Exp))
        acts.append(nc.scalar.activation(ets[0][:], halves[0][:], AFT.Exp))
        acts.append(nc.scalar.activation(ets[1][:], halves[1][:], AFT.Exp))

        # --- top-8 per compacted gm-group on exp-domain values ---
        top8 = pool.tile([parts, G * 8], f32)
        for g in range(G):
            nc.vector.max(top8[:, g * 8:(g + 1) * 8],
                          ge[:, g * 16:(g + 1) * 16])
        t8e = top8[:].rearrange("p (g e) -> p g e", e=8)
        # pn_cat = [1/exp(pos) , exp(neg)] -> Ln(1+.) gives sp(-pos), sp(neg)
        pn_cat = pool.tile([parts, 2 * G], f32)
        dves.append(nc.vector.reciprocal(pn_cat[:, :G], t8e[:, :, 1]))
        dves.append(nc.vector.tensor_copy(pn_cat[:, G:], t8e[:, :, 2]))

        # --- ln-set phase ---
        pn_ln = pool.tile([parts, 2 * G], f32)
        acts.append(nc.scalar.activation(pn_ln[:], pn_cat[:], AFT.Ln, bias=1.0))
        for h in range(2):
            acts.append(nc.scalar.activation(sps[h][:], ets[h][:], AFT.Ln,
                                             bias=1.0))

        # --- raw partial sums into accs columns; host applies the scales ---
        accs = pool.tile([parts, 6], f32)
        dves.append(nc.vector.tensor_reduce(
            accs[:, 2:3], xmt[:].rearrange("p (k l) -> p k l", l=L),
            axis=AX.XY, op=ALU.add))
        dves.append(nc.vector.tensor_reduce(accs[:, 3:4], sps[0][:],
                                            axis=AX.X, op=ALU.add))
        dves.append(nc.vector.tensor_reduce(accs[:, 4:5], sps[1][:],
                                            axis=AX.X, op=ALU.add))
        dves.append(nc.vector.tensor_reduce(accs[:, 0:1], pn_ln[:, :G],
                                            axis=AX.X, op=ALU.add))
        dves.append(nc.vector.tensor_reduce(accs[:, 1:2], pn_ln[:, G:],
                                            axis=AX.X, op=ALU.add))
        dves.append(nc.vector.memset(accs[:, 5:6], 0.0))
        nc.sync.dma_start(out_d[:], accs[:], single_packet=True)

        # pin ScalarE program order: all exp-set work, then all ln-set work
        chain(acts)
        chain(dves)

    if os.environ.get('ATL_PATCH', '0') == '1':
        _prefer_combined_act_table()
    nc.compile()
    return nc


_ACT_PATCHED = False


def _prefer_combined_act_table():
    """Bias bacc's table chooser toward the set holding both Exp and Ln so a
    single ACT_TABLE_LOAD serves the whole kernel."""
    global _ACT_PATCHED
    if _ACT_PATCHED:
        return
    import concourse.bacc as bacc_mod
    import concourse.hw_specs as hw_specs_mod

    orig = hw_specs_mod.get_activation_tables

    def _patched(arch):
        tabs = orig(arch)
        pref = "natural_log_exp_and_others"
        if pref in tabs:
            out = {pref: tabs[pref]}
            out.update({k: v for k, v in tabs.items() if k != pref})
            return out
        return tabs

    bacc_mod.get_activation_tables = _patched
    _ACT_PATCHED = True


# ---------------------------------------------------------------- entrypoint
def _run(pred_logits, points, knn_indices, gt_triangles, **run_kwargs):
    from concourse.bass_utils import run_bass_kernel_spmd

    x = np.ascontiguousarray(np.asarray(pred_logits, dtype=np.float32))
    gt_mask, gm, w, inv_denom, inv_cnt = _host_prep(
        pred_logits, points, knn_indices, gt_triangles)
    in_maps, L, g_chunks, cap_pc = _make_shards(x, gt_mask, gm, w,
                                                 inv_denom, inv_cnt)
    nc = _build_bass(L, g_chunks, cap_pc)
    res = run_bass_kernel_spmd(nc, in_maps, core_ids=list(range(N_CORES)),
                               **run_kwargs)
    acc = np.zeros(6, np.float64)
    for r in res.results:
        acc += np.asarray(r["out"], dtype=np.float64).reshape(128, 6).sum(axis=0)
    pos_t, neg_t, xm_t = acc[0], acc[1], acc[2]
    sp_t = acc[3] + acc[4]
    total = np.array([(sp_t - xm_t) * float(inv_denom),
                      pos_t * float(inv_cnt),
                      neg_t * float(inv_cnt)])
    return total.astype(np.float32), res


def kernel(pred_logits, points, knn_indices, gt_triangles):
    out, _ = _run(pred_logits, points, knn_indices, gt_triangles)
    return out


# revision 40
# speedup vs baseline: 1.0901x; 1.0901x over previous
"""Trainium2 Bass kernel for nn_BinaryLoss (BCE triangle-mesh loss).

Structure
---------
Host (integer combinatorics on the tiny index tensors only; no FP math on
logits): sorted-triangle key table -> unique keys; undirected GT edge set;
per-vertex unique-triangle counts; candidate-triple membership gt_mask
[N,256] via searchsorted; manifold row mask w [N]; edge mask gm [N,16].
Two exact identities drive the device plan:
  * gt_labels_masked == gt_mask (a GT triangle always contributes its own
    (e0,e1) edge to full_mat, so the dense adjacency lookup is redundant),
  * sum_m [sp(x) - x*mask] needs only softplus sums plus the sum of x over
    masked positions (<= 8 per row here, gathered to a narrow [rows,L]).
Only manifold rows (w==1, ~800 of 16384) contribute to the main loss, so
just those rows' logits ship to the device.

Device (all logit FP math, 8 cores data-parallel, per core):
  * gsel = compacted gm==1 groups of 16 logits, exp()'d on ScalarE
    (monotonic, ranks unchanged) then the DVE Max8 instruction gives the
    exact descending top-8 per group: rank1=exp(pos), rank2=exp(neg).
    sp(-pos)=Ln(1+1/exp(pos)) via DVE reciprocal + Ln, sp(neg)=Ln(1+exp(neg))
    -- no exp needed after the single activation-table switch.
  * selected rows: softplus via Exp then Ln(1+e) on ScalarE (transposed
    layout), per-partition sums on DVE.
  * ScalarE/DVE instruction orders are pinned with explicit dep edges to
    keep one exp->ln table transition and a stall-free DVE tail.
  * per-core raw partial sums [128,6] DMA out; the host applies inv_denom /
    inv_cnt and does the cross-core/partition scalar all-reduce.
Pad rows/groups use +-30 logits so their softplus terms are ~1e-13.
"""
import os
import numpy as np

N_CORES = 8
B_PAD = 30.0  # pad-group magnitude: softplus(-30) ~ 9e-14


# ---------------------------------------------------------------- host prep
def _host_prep(pred_logits, points, knn_indices, gt_triangles):
    N, K = knn_indices.shape
    M = (K - 1) * (K - 1)
    num_pts = points.shape[0]
    P = num_pts + 1

    tri = np.sort(np.asarray(gt_triangles, dtype=np.int64), axis=1)
    keys = tri[:, 0] * (P * P) + tri[:, 1] * P + tri[:, 2]
    uk = np.unique(keys)

    ut0, ut1, ut2 = uk // (P * P), (uk // P) % P, uk % P
    counts = np.zeros(P, np.float64)
    np.add.at(counts, ut0, 1.0)
    np.add.at(counts, ut1, (ut1 != ut0).astype(np.float64))
    np.add.at(counts, ut2, (ut2 != ut1).astype(np.float64))
    all_N_gt = counts[np.asarray(knn_indices[:, 0], dtype=np.int64)]

    e_u = np.concatenate([np.minimum(tri[:, 0], tri[:, 1]),
                          np.minimum(tri[:, 1], tri[:, 2]),
                          np.minimum(tri[:, 0], tri[:, 2])])
    e_v = np.concatenate([np.maximum(tri[:, 0], tri[:, 1]),
                          np.maximum(tri[:, 1], tri[:, 2]),
                          np.maximum(tri[:, 0], tri[:, 2])])
    ekeys = np.unique(e_u * P + e_v)

    c = np.asarray(knn_indices[:, 0], dtype=np.int64)[:, None]
    a = np.asarray(knn_indices[:, 1:], dtype=np.int64)
    q = np.minimum(c, a) * P + np.maximum(c, a)
    pos = np.clip(np.searchsorted(ekeys, q.ravel()), 0, len(ekeys) - 1)
    gm = (ekeys[pos] == q.ravel()).reshape(N, K - 1)

    e0 = np.repeat(a, K - 1, axis=1)
    e1 = np.tile(a, (1, K - 1))
    v0 = np.broadcast_to(c, e0.shape)
    cand = np.stack([v0, e0, e1], axis=-1)
    cand.sort(axis=-1)
    ck = cand[..., 0] * (P * P) + cand[..., 1] * P + cand[..., 2]
    cpos = np.clip(np.searchsorted(uk, ck.ravel()), 0, len(uk) - 1)
    gt_mask = (uk[cpos] == ck.ravel()).reshape(N, M)

    all_N_pred = gt_mask.sum(1).astype(np.float64)
    manifold = (all_N_gt * 2.0) == all_N_pred
    w = manifold.astype(np.float32)

    inv_denom = np.float32(1.0 / max(float(w.sum(dtype=np.float64)) * M, 1.0))
    inv_cnt = np.float32(1.0 / max(float(gm.sum(dtype=np.float64)), 1.0))
    return gt_mask, gm, w, inv_denom, inv_cnt


def _make_shards(x, gt_mask, gm, w, inv_denom, inv_cnt):
    """Build per-core input dicts. x is [N,256] f32."""
    N, M = x.shape
    parts = 128

    # masked-x values padded to L per row (L chosen from data)
    mask_per_row = gt_mask.sum(1)
    L = max(8, int(mask_per_row.max()))
    L = int(2 ** np.ceil(np.log2(L)))
    rr, cc = np.nonzero(gt_mask)
    xm = np.zeros((N, L), np.float32)
    row_starts = np.zeros(N + 1, np.int64)
    np.add.at(row_starts, rr + 1, 1)
    row_starts = np.cumsum(row_starts)
    ranks = np.arange(len(rr)) - row_starts[rr]
    xm[rr, ranks] = x[rr, cc]

    # only manifold rows (w==1) contribute to the main BCE: select them
    sel = np.nonzero(w)[0]
    W = len(sel)
    cap_pc = max(parts, int(np.ceil(W / (N_CORES * parts))) * parts)
    CAP = cap_pc * N_CORES
    xs = np.full((CAP, M), -B_PAD, np.float32)   # pad rows: softplus ~ 1e-13
    xs[:W] = x[sel]
    xms = np.zeros((CAP, L), np.float32)
    xms[:W] = xm[sel]

    # compacted gm groups, padded; distributed evenly over cores
    gn, gi = np.nonzero(gm)               # group ids (row, i)
    total = len(gn)
    per_core = int(np.ceil(total / N_CORES))
    g_chunks = max(1, int(np.ceil(per_core / parts)))  # free-dim group chunks
    cap = g_chunks * parts                       # groups per core
    pl3 = x.reshape(N, 16, 16)

    pad_group = np.full(16, -B_PAD, np.float32)
    pad_group[0] = B_PAD
    pad_group[1] = B_PAD

    in_maps = []
    for core in range(N_CORES):
        s0, s1 = core * cap_pc, (core + 1) * cap_pc
        xc = np.ascontiguousarray(xs[s0:s1].T)          # [256, cap_pc] f32
        kk = cap_pc // parts
        xmc = np.ascontiguousarray(xms[s0:s1]).reshape(parts, kk * L)

        lo, hi = core * per_core, min((core + 1) * per_core, total)
        gsel = np.broadcast_to(pad_group, (cap, 16)).copy()
        if hi > lo:
            gsel[: hi - lo] = pl3[gn[lo:hi], gi[lo:hi], :]
        gsel = np.ascontiguousarray(
            gsel.reshape(g_chunks, parts, 16).transpose(1, 0, 2)
        ).reshape(parts, g_chunks * 16)

        in_maps.append({"x": xc, "xm": xmc, "gsel": gsel})
    return in_maps, L, g_chunks, cap_pc


# ---------------------------------------------------------------- bass build
def _build_bass(L, g_chunks, cap_pc):
    from contextlib import ExitStack

    import concourse.bacc as bacc
    import concourse.mybir as mybir
    import concourse.tile as tile

    f32 = mybir.dt.float32
    bf16 = mybir.dt.bfloat16
    AFT = mybir.ActivationFunctionType
    ALU = mybir.AluOpType
    AX = mybir.AxisListType

    parts, rpp = 128, 16
    G = g_chunks
    S = cap_pc          # selected rows per core
    KK = S // parts     # xm row-chunks per partition

    nc = bacc.Bacc(
        "TRN2", target_bir_lowering=False, debug=False,
        enable_asserts=False, num_devices=N_CORES,
    )
    x_d = nc.dram_tensor("x", [2 * parts, S], f32, kind="ExternalInput").ap()
    xm_d = nc.dram_tensor("xm", [parts, KK * L], f32, kind="ExternalInput").ap()
    g_d = nc.dram_tensor("gsel", [parts, G * 16], f32, kind="ExternalInput").ap()
    out_d = nc.dram_tensor("out", [128, 6], f32, kind="ExternalOutput").ap()

    with tile.TileContext(nc) as tc, ExitStack() as ctx:
        from concourse.tile import add_dep_helper

        def chain(lst):
            for a, b in zip(lst, lst[1:]):
                add_dep_helper(b.ins, a.ins, sync=True, reason="engine order")

        pool = ctx.enter_context(tc.tile_pool(name="main", bufs=1))

        acts = []  # explicit ScalarE program order (avoids table-load thrash)
        dves = []  # pinned DVE order for the post-Max8 tail

        # hoist the exp-table load: dummy activation with no DMA deps
        dumt = pool.tile([1, 8], f32)
        nc.vector.memset(dumt[:], 0.0)
        dumo = pool.tile([1, 8], f32)
        acts.append(nc.scalar.activation(dumo[:], dumt[:], AFT.Exp))

        # --- DMAs: gsel first (feeds the critical Max8 chain) ---
        gt = pool.tile([parts, G * 16], f32)
        NGC = 4
        gch = G * 16 // NGC
        for i in range(NGC):
            nc.sync.dma_start(gt[:, i * gch:(i + 1) * gch],
                              g_d[:, i * gch:(i + 1) * gch])
        halves = []
        for h in range(2):
            xth = pool.tile([parts, S], f32, name=f"xt{h}", tag=f"xt{h}")
            nc.gpsimd.dma_start(xth[:], x_d[h * parts:(h + 1) * parts, :])
            halves.append(xth)
        xmt = pool.tile([parts, KK * L], f32)
        nc.gpsimd.dma_start(xmt[:], xm_d[:])

        # --- exp over gsel (monotonic: Max8 ranks unchanged) interleaved
        #     with the selected-row exps so the ln table switch lands early
        ge = pool.tile([parts, G * 16], f32)
        ets, sps = [], []
        for h in range(2):
            ets.append(pool.tile([parts, S], f32, name=f"e{h}", tag=f"e{h}"))
            sps.append(pool.tile([parts, S], f32, name=f"sp{h}", tag=f"sp{h}"))
        for i in range(2):
            acts.append(nc.scalar.activation(ge[:, i * gch:(i + 1) * gch],
                                             gt[:, i * gch:(i + 1) * gch],
                                             AFT.Exp))
        acts.append(nc.scalar.activation(ets[0][:], halves[0][:], AFT.Exp))
        for i in range(2, NGC):
            acts.append(nc.scalar.activation(ge[:, i * gch:(i + 1) * gch],
                                             gt[:, i * gch:(i + 1) * gch],
                                             AFT.Exp))
        acts.append(nc.scalar.activation(ets[1][:], halves[1][:], AFT.Exp))

        # --- top-8 per compacted gm-group on exp-domain values ---
        top8 = pool.tile([parts, G * 8], f32)
        for g in range(G):
            nc.vector.max(top8[:, g * 8:(g + 1) * 8],
                          ge[:, g * 16:(g + 1) * 16])
        t8e = top8[:].rearrange("p (g e) -> p g e", e=8)
        # pn_cat = [1/exp(pos) , exp(neg)] -> Ln(1+.) gives sp(-pos), sp(neg)
        pn_cat = pool.tile([parts, 2 * G], f32)
        dves.append(nc.vector.reciprocal(pn_cat[:, :G], t8e[:, :, 1]))
        dves.append(nc.vector.tensor_copy(pn_cat[:, G:], t8e[:, :, 2]))

        # --- ln-set phase ---
        for h in range(2):
            acts.append(nc.scalar.activation(sps[h][:], ets[h][:], AFT.Ln,
                                             bias=1.0))
        pn_ln = pool.tile([parts, 2 * G], f32)
        acts.append(nc.scalar.activation(pn_ln[:], pn_cat[:], AFT.Ln, bias=1.0))

        # --- raw partial sums into accs columns; host applies the scales ---
        accs = pool.tile([parts, 6], f32)
        dves.append(nc.vector.tensor_reduce(
            accs[:, 2:3], xmt[:].rearrange("p (k l) -> p k l", l=L),
            axis=AX.XY, op=ALU.add))
        dves.append(nc.vector.tensor_reduce(accs[:, 3:4], sps[0][:],
                                            axis=AX.X, op=ALU.add))
        dves.append(nc.vector.tensor_reduce(accs[:, 4:5], sps[1][:],
                                            axis=AX.X, op=ALU.add))
        dves.append(nc.vector.tensor_reduce(accs[:, 0:1], pn_ln[:, :G],
                                            axis=AX.X, op=ALU.add))
        dves.append(nc.vector.tensor_reduce(accs[:, 1:2], pn_ln[:, G:],
                                            axis=AX.X, op=ALU.add))
        dves.append(nc.vector.memset(accs[:, 5:6], 0.0))
        nc.sync.dma_start(out_d[:], accs[:])

        # pin ScalarE program order: all exp-set work, then all ln-set work
        chain(acts)
        chain(dves)

    if os.environ.get('ATL_PATCH', '0') == '1':
        _prefer_combined_act_table()
    nc.compile()
    return nc


_ACT_PATCHED = False


def _prefer_combined_act_table():
    """Bias bacc's table chooser toward the set holding both Exp and Ln so a
    single ACT_TABLE_LOAD serves the whole kernel."""
    global _ACT_PATCHED
    if _ACT_PATCHED:
        return
    import concourse.bacc as bacc_mod
    import concourse.hw_specs as hw_specs_mod

    orig = hw_specs_mod.get_activation_tables

    def _patched(arch):
        tabs = orig(arch)
        pref = "natural_log_exp_and_others"
        if pref in tabs:
            out = {pref: tabs[pref]}
            out.update({k: v for k, v in tabs.items() if k != pref})
            return out
        return tabs

    bacc_mod.get_activation_tables = _patched
    _ACT_PATCHED = True


# ---------------------------------------------------------------- entrypoint
def _run(pred_logits, points, knn_indices, gt_triangles, **run_kwargs):
    from concourse.bass_utils import run_bass_kernel_spmd

    x = np.ascontiguousarray(np.asarray(pred_logits, dtype=np.float32))
    gt_mask, gm, w, inv_denom, inv_cnt = _host_prep(
        pred_logits, points, knn_indices, gt_triangles)
    in_maps, L, g_chunks, cap_pc = _make_shards(x, gt_mask, gm, w,
                                                 inv_denom, inv_cnt)
    nc = _build_bass(L, g_chunks, cap_pc)
    res = run_bass_kernel_spmd(nc, in_maps, core_ids=list(range(N_CORES)),
                               **run_kwargs)
    acc = np.zeros(6, np.float64)
    for r in res.results:
        acc += np.asarray(r["out"], dtype=np.float64).reshape(128, 6).sum(axis=0)
    pos_t, neg_t, xm_t = acc[0], acc[1], acc[2]
    sp_t = acc[3] + acc[4]
    total = np.array([(sp_t - xm_t) * float(inv_denom),
                      pos_t * float(inv_cnt),
                      neg_t * float(inv_cnt)])
    return total.astype(np.float32), res


def kernel(pred_logits, points, knn_indices, gt_triangles):
    out, _ = _run(pred_logits, points, knn_indices, gt_triangles)
    return out


# revision 41
# speedup vs baseline: 1.1148x; 1.0227x over previous
"""Trainium2 Bass kernel for nn_BinaryLoss (BCE triangle-mesh loss).

Structure
---------
Host (integer combinatorics on the tiny index tensors only; no FP math on
logits): sorted-triangle key table -> unique keys; undirected GT edge set;
per-vertex unique-triangle counts; candidate-triple membership gt_mask
[N,256] via searchsorted; manifold row mask w [N]; edge mask gm [N,16].
Two exact identities drive the device plan:
  * gt_labels_masked == gt_mask (a GT triangle always contributes its own
    (e0,e1) edge to full_mat, so the dense adjacency lookup is redundant),
  * sum_m [sp(x) - x*mask] needs only softplus sums plus the sum of x over
    masked positions (<= 8 per row here, gathered to a narrow [rows,L]).
Only manifold rows (w==1, ~800 of 16384) contribute to the main loss, so
just those rows' logits ship to the device.

Device (all logit FP math, 8 cores data-parallel, per core):
  * gsel = compacted gm==1 groups of 16 logits, exp()'d on ScalarE
    (monotonic, ranks unchanged) then the DVE Max8 instruction gives the
    exact descending top-8 per group: rank1=exp(pos), rank2=exp(neg).
    sp(-pos)=Ln(1+1/exp(pos)) via DVE reciprocal + Ln, sp(neg)=Ln(1+exp(neg))
    -- no exp needed after the single activation-table switch.
  * selected rows: softplus via Exp then Ln(1+e) on ScalarE (transposed
    layout), per-partition sums on DVE.
  * ScalarE/DVE instruction orders are pinned with explicit dep edges to
    keep one exp->ln table transition and a stall-free DVE tail.
  * per-core raw partial sums [128,6] DMA out; the host applies inv_denom /
    inv_cnt and does the cross-core/partition scalar all-reduce.
Pad rows/groups use +-30 logits so their softplus terms are ~1e-13.
"""
import os
import numpy as np

N_CORES = 8
B_PAD = 30.0  # pad-group magnitude: softplus(-30) ~ 9e-14


# ---------------------------------------------------------------- host prep
def _host_prep(pred_logits, points, knn_indices, gt_triangles):
    N, K = knn_indices.shape
    M = (K - 1) * (K - 1)
    num_pts = points.shape[0]
    P = num_pts + 1

    tri = np.sort(np.asarray(gt_triangles, dtype=np.int64), axis=1)
    keys = tri[:, 0] * (P * P) + tri[:, 1] * P + tri[:, 2]
    uk = np.unique(keys)

    ut0, ut1, ut2 = uk // (P * P), (uk // P) % P, uk % P
    counts = np.zeros(P, np.float64)
    np.add.at(counts, ut0, 1.0)
    np.add.at(counts, ut1, (ut1 != ut0).astype(np.float64))
    np.add.at(counts, ut2, (ut2 != ut1).astype(np.float64))
    all_N_gt = counts[np.asarray(knn_indices[:, 0], dtype=np.int64)]

    e_u = np.concatenate([np.minimum(tri[:, 0], tri[:, 1]),
                          np.minimum(tri[:, 1], tri[:, 2]),
                          np.minimum(tri[:, 0], tri[:, 2])])
    e_v = np.concatenate([np.maximum(tri[:, 0], tri[:, 1]),
                          np.maximum(tri[:, 1], tri[:, 2]),
                          np.maximum(tri[:, 0], tri[:, 2])])
    ekeys = np.unique(e_u * P + e_v)

    c = np.asarray(knn_indices[:, 0], dtype=np.int64)[:, None]
    a = np.asarray(knn_indices[:, 1:], dtype=np.int64)
    q = np.minimum(c, a) * P + np.maximum(c, a)
    pos = np.clip(np.searchsorted(ekeys, q.ravel()), 0, len(ekeys) - 1)
    gm = (ekeys[pos] == q.ravel()).reshape(N, K - 1)

    e0 = np.repeat(a, K - 1, axis=1)
    e1 = np.tile(a, (1, K - 1))
    v0 = np.broadcast_to(c, e0.shape)
    cand = np.stack([v0, e0, e1], axis=-1)
    cand.sort(axis=-1)
    ck = cand[..., 0] * (P * P) + cand[..., 1] * P + cand[..., 2]
    cpos = np.clip(np.searchsorted(uk, ck.ravel()), 0, len(uk) - 1)
    gt_mask = (uk[cpos] == ck.ravel()).reshape(N, M)

    all_N_pred = gt_mask.sum(1).astype(np.float64)
    manifold = (all_N_gt * 2.0) == all_N_pred
    w = manifold.astype(np.float32)

    inv_denom = np.float32(1.0 / max(float(w.sum(dtype=np.float64)) * M, 1.0))
    inv_cnt = np.float32(1.0 / max(float(gm.sum(dtype=np.float64)), 1.0))
    return gt_mask, gm, w, inv_denom, inv_cnt


def _make_shards(x, gt_mask, gm, w, inv_denom, inv_cnt):
    """Build per-core input dicts. x is [N,256] f32."""
    N, M = x.shape
    parts = 128

    # masked-x values padded to L per row (L chosen from data)
    mask_per_row = gt_mask.sum(1)
    L = max(8, int(mask_per_row.max()))
    L = int(2 ** np.ceil(np.log2(L)))
    rr, cc = np.nonzero(gt_mask)
    xm = np.zeros((N, L), np.float32)
    row_starts = np.zeros(N + 1, np.int64)
    np.add.at(row_starts, rr + 1, 1)
    row_starts = np.cumsum(row_starts)
    ranks = np.arange(len(rr)) - row_starts[rr]
    xm[rr, ranks] = x[rr, cc]

    # only manifold rows (w==1) contribute to the main BCE: select them
    sel = np.nonzero(w)[0]
    W = len(sel)
    cap_pc = max(parts, int(np.ceil(W / (N_CORES * parts))) * parts)
    CAP = cap_pc * N_CORES
    xs = np.full((CAP, M), -B_PAD, np.float32)   # pad rows: softplus ~ 1e-13
    xs[:W] = x[sel]
    xms = np.zeros((CAP, L), np.float32)
    xms[:W] = xm[sel]

    # compacted gm groups, padded; distributed evenly over cores
    gn, gi = np.nonzero(gm)               # group ids (row, i)
    total = len(gn)
    per_core = int(np.ceil(total / N_CORES))
    g_chunks = max(1, int(np.ceil(per_core / parts)))  # free-dim group chunks
    cap = g_chunks * parts                       # groups per core
    pl3 = x.reshape(N, 16, 16)

    pad_group = np.full(16, -B_PAD, np.float32)
    pad_group[0] = B_PAD
    pad_group[1] = B_PAD

    in_maps = []
    for core in range(N_CORES):
        s0, s1 = core * cap_pc, (core + 1) * cap_pc
        xt_ = xs[s0:s1].T                               # [256, cap_pc]
        xc = np.ascontiguousarray(
            np.concatenate([xt_[:128], xt_[128:]], axis=1))  # [128, 2*cap_pc]
        kk = cap_pc // parts
        xmc = np.ascontiguousarray(xms[s0:s1]).reshape(parts, kk * L)

        lo, hi = core * per_core, min((core + 1) * per_core, total)
        gsel = np.broadcast_to(pad_group, (cap, 16)).copy()
        if hi > lo:
            gsel[: hi - lo] = pl3[gn[lo:hi], gi[lo:hi], :]
        gsel = np.ascontiguousarray(
            gsel.reshape(g_chunks, parts, 16).transpose(1, 0, 2)
        ).reshape(parts, g_chunks * 16)

        in_maps.append({"x": xc, "xm": xmc, "gsel": gsel})
    return in_maps, L, g_chunks, cap_pc


# ---------------------------------------------------------------- bass build
def _build_bass(L, g_chunks, cap_pc):
    from contextlib import ExitStack

    import concourse.bacc as bacc
    import concourse.mybir as mybir
    import concourse.tile as tile

    f32 = mybir.dt.float32
    bf16 = mybir.dt.bfloat16
    AFT = mybir.ActivationFunctionType
    ALU = mybir.AluOpType
    AX = mybir.AxisListType

    parts, rpp = 128, 16
    G = g_chunks
    S = cap_pc          # selected rows per core
    KK = S // parts     # xm row-chunks per partition

    nc = bacc.Bacc(
        "TRN2", target_bir_lowering=False, debug=False,
        enable_asserts=False, num_devices=N_CORES,
    )
    x_d = nc.dram_tensor("x", [parts, 2 * S], f32, kind="ExternalInput").ap()
    xm_d = nc.dram_tensor("xm", [parts, KK * L], f32, kind="ExternalInput").ap()
    g_d = nc.dram_tensor("gsel", [parts, G * 16], f32, kind="ExternalInput").ap()
    out_d = nc.dram_tensor("out", [128, 6], f32, kind="ExternalOutput").ap()

    with tile.TileContext(nc) as tc, ExitStack() as ctx:
        from concourse.tile import add_dep_helper

        def chain(lst):
            for a, b in zip(lst, lst[1:]):
                add_dep_helper(b.ins, a.ins, sync=True, reason="engine order")

        pool = ctx.enter_context(tc.tile_pool(name="main", bufs=1))

        acts = []  # explicit ScalarE program order (avoids table-load thrash)
        dves = []  # pinned DVE order for the post-Max8 tail

        # hoist the exp-table load: dummy activation with no DMA deps
        dumt = pool.tile([1, 8], f32)
        nc.vector.memset(dumt[:], 0.0)
        dumo = pool.tile([1, 8], f32)
        acts.append(nc.scalar.activation(dumo[:], dumt[:], AFT.Exp))

        # --- DMAs: gsel first (feeds the critical Max8 chain) ---
        gt = pool.tile([parts, G * 16], f32)
        NGC = 4
        gch = G * 16 // NGC
        for i in range(NGC):
            nc.sync.dma_start(gt[:, i * gch:(i + 1) * gch],
                              g_d[:, i * gch:(i + 1) * gch])
        xta = pool.tile([parts, 2 * S], f32)
        nc.gpsimd.dma_start(xta[:], x_d[:])
        xmt = pool.tile([parts, KK * L], f32)
        nc.gpsimd.dma_start(xmt[:], xm_d[:])

        # --- exp over gsel (monotonic: Max8 ranks unchanged) interleaved
        #     with the selected-row exps so the ln table switch lands early
        ge = pool.tile([parts, G * 16], f32)
        eta = pool.tile([parts, 2 * S], f32)
        spa = pool.tile([parts, 2 * S], f32)
        for i in range(2):
            acts.append(nc.scalar.activation(ge[:, i * gch:(i + 1) * gch],
                                             gt[:, i * gch:(i + 1) * gch],
                                             AFT.Exp))
        acts.append(nc.scalar.activation(eta[:], xta[:], AFT.Exp))
        for i in range(2, NGC):
            acts.append(nc.scalar.activation(ge[:, i * gch:(i + 1) * gch],
                                             gt[:, i * gch:(i + 1) * gch],
                                             AFT.Exp))

        # --- top-8 per compacted gm-group on exp-domain values ---
        top8 = pool.tile([parts, G * 8], f32)
        for g in range(G):
            nc.vector.max(top8[:, g * 8:(g + 1) * 8],
                          ge[:, g * 16:(g + 1) * 16])
        t8e = top8[:].rearrange("p (g e) -> p g e", e=8)
        # pn_cat = [1/exp(pos) , exp(neg)] -> Ln(1+.) gives sp(-pos), sp(neg)
        pn_cat = pool.tile([parts, 2 * G], f32)
        dves.append(nc.vector.reciprocal(pn_cat[:, :G], t8e[:, :, 1]))
        dves.append(nc.vector.tensor_copy(pn_cat[:, G:], t8e[:, :, 2]))

        # --- ln-set phase ---
        acts.append(nc.scalar.activation(spa[:], eta[:], AFT.Ln, bias=1.0))
        pn_ln = pool.tile([parts, 2 * G], f32)
        acts.append(nc.scalar.activation(pn_ln[:], pn_cat[:], AFT.Ln, bias=1.0))

        # --- raw partial sums into accs columns; host applies the scales ---
        accs = pool.tile([parts, 6], f32)
        dves.append(nc.vector.tensor_reduce(
            accs[:, 2:3], xmt[:].rearrange("p (k l) -> p k l", l=L),
            axis=AX.XY, op=ALU.add))
        dves.append(nc.vector.tensor_reduce(accs[:, 3:4], spa[:],
                                            axis=AX.X, op=ALU.add))
        dves.append(nc.vector.memset(accs[:, 4:5], 0.0))
        dves.append(nc.vector.tensor_reduce(accs[:, 0:1], pn_ln[:, :G],
                                            axis=AX.X, op=ALU.add))
        dves.append(nc.vector.tensor_reduce(accs[:, 1:2], pn_ln[:, G:],
                                            axis=AX.X, op=ALU.add))
        dves.append(nc.vector.memset(accs[:, 5:6], 0.0))
        nc.sync.dma_start(out_d[:], accs[:])

        # pin ScalarE program order: all exp-set work, then all ln-set work
        chain(acts)
        chain(dves)

    if os.environ.get('ATL_PATCH', '0') == '1':
        _prefer_combined_act_table()
    nc.compile()
    return nc


_ACT_PATCHED = False


def _prefer_combined_act_table():
    """Bias bacc's table chooser toward the set holding both Exp and Ln so a
    single ACT_TABLE_LOAD serves the whole kernel."""
    global _ACT_PATCHED
    if _ACT_PATCHED:
        return
    import concourse.bacc as bacc_mod
    import concourse.hw_specs as hw_specs_mod

    orig = hw_specs_mod.get_activation_tables

    def _patched(arch):
        tabs = orig(arch)
        pref = "natural_log_exp_and_others"
        if pref in tabs:
            out = {pref: tabs[pref]}
            out.update({k: v for k, v in tabs.items() if k != pref})
            return out
        return tabs

    bacc_mod.get_activation_tables = _patched
    _ACT_PATCHED = True


# ---------------------------------------------------------------- entrypoint
def _run(pred_logits, points, knn_indices, gt_triangles, **run_kwargs):
    from concourse.bass_utils import run_bass_kernel_spmd

    x = np.ascontiguousarray(np.asarray(pred_logits, dtype=np.float32))
    gt_mask, gm, w, inv_denom, inv_cnt = _host_prep(
        pred_logits, points, knn_indices, gt_triangles)
    in_maps, L, g_chunks, cap_pc = _make_shards(x, gt_mask, gm, w,
                                                 inv_denom, inv_cnt)
    nc = _build_bass(L, g_chunks, cap_pc)
    res = run_bass_kernel_spmd(nc, in_maps, core_ids=list(range(N_CORES)),
                               **run_kwargs)
    acc = np.zeros(6, np.float64)
    for r in res.results:
        acc += np.asarray(r["out"], dtype=np.float64).reshape(128, 6).sum(axis=0)
    pos_t, neg_t, xm_t = acc[0], acc[1], acc[2]
    sp_t = acc[3] + acc[4]
    total = np.array([(sp_t - xm_t) * float(inv_denom),
                      pos_t * float(inv_cnt),
                      neg_t * float(inv_cnt)])
    return total.astype(np.float32), res


def kernel(pred_logits, points, knn_indices, gt_triangles):
    out, _ = _run(pred_logits, points, knn_indices, gt_triangles)
    return out


# revision 42
# speedup vs baseline: 1.1446x; 1.0267x over previous
"""Trainium2 Bass kernel for nn_BinaryLoss (BCE triangle-mesh loss).

Structure
---------
Host (integer combinatorics on the tiny index tensors only; no FP math on
logits): sorted-triangle key table -> unique keys; undirected GT edge set;
per-vertex unique-triangle counts; candidate-triple membership gt_mask
[N,256] via searchsorted; manifold row mask w [N]; edge mask gm [N,16].
Two exact identities drive the device plan:
  * gt_labels_masked == gt_mask (a GT triangle always contributes its own
    (e0,e1) edge to full_mat, so the dense adjacency lookup is redundant),
  * sum_m [sp(x) - x*mask] needs only softplus sums plus the sum of x over
    masked positions (<= 8 per row here, gathered to a narrow [rows,L]).
Only manifold rows (w==1, ~800 of 16384) contribute to the main loss, so
just those rows' logits ship to the device.

Device (all logit FP math, 8 cores data-parallel, per core):
  * gsel = compacted gm==1 groups of 16 logits, exp()'d on ScalarE
    (monotonic, ranks unchanged) then the DVE Max8 instruction gives the
    exact descending top-8 per group: rank1=exp(pos), rank2=exp(neg).
    sp(-pos)=Ln(1+1/exp(pos)) via DVE reciprocal + Ln, sp(neg)=Ln(1+exp(neg))
    -- no exp needed after the single activation-table switch.
  * selected rows: softplus via Exp then Ln(1+e) on ScalarE (transposed
    layout), per-partition sums on DVE.
  * ScalarE/DVE instruction orders are pinned with explicit dep edges to
    keep one exp->ln table transition and a stall-free DVE tail.
  * per-core raw partial sums [128,6] DMA out; the host applies inv_denom /
    inv_cnt and does the cross-core/partition scalar all-reduce.
Pad rows/groups use +-30 logits so their softplus terms are ~1e-13.
"""
import os
import numpy as np

N_CORES = 8
B_PAD = 30.0  # pad-group magnitude: softplus(-30) ~ 9e-14


# ---------------------------------------------------------------- host prep
def _host_prep(pred_logits, points, knn_indices, gt_triangles):
    N, K = knn_indices.shape
    M = (K - 1) * (K - 1)
    num_pts = points.shape[0]
    P = num_pts + 1

    tri = np.sort(np.asarray(gt_triangles, dtype=np.int64), axis=1)
    keys = tri[:, 0] * (P * P) + tri[:, 1] * P + tri[:, 2]
    uk = np.unique(keys)

    ut0, ut1, ut2 = uk // (P * P), (uk // P) % P, uk % P
    counts = np.zeros(P, np.float64)
    np.add.at(counts, ut0, 1.0)
    np.add.at(counts, ut1, (ut1 != ut0).astype(np.float64))
    np.add.at(counts, ut2, (ut2 != ut1).astype(np.float64))
    all_N_gt = counts[np.asarray(knn_indices[:, 0], dtype=np.int64)]

    e_u = np.concatenate([np.minimum(tri[:, 0], tri[:, 1]),
                          np.minimum(tri[:, 1], tri[:, 2]),
                          np.minimum(tri[:, 0], tri[:, 2])])
    e_v = np.concatenate([np.maximum(tri[:, 0], tri[:, 1]),
                          np.maximum(tri[:, 1], tri[:, 2]),
                          np.maximum(tri[:, 0], tri[:, 2])])
    ekeys = np.unique(e_u * P + e_v)

    c = np.asarray(knn_indices[:, 0], dtype=np.int64)[:, None]
    a = np.asarray(knn_indices[:, 1:], dtype=np.int64)
    q = np.minimum(c, a) * P + np.maximum(c, a)
    pos = np.clip(np.searchsorted(ekeys, q.ravel()), 0, len(ekeys) - 1)
    gm = (ekeys[pos] == q.ravel()).reshape(N, K - 1)

    e0 = np.repeat(a, K - 1, axis=1)
    e1 = np.tile(a, (1, K - 1))
    v0 = np.broadcast_to(c, e0.shape)
    cand = np.stack([v0, e0, e1], axis=-1)
    cand.sort(axis=-1)
    ck = cand[..., 0] * (P * P) + cand[..., 1] * P + cand[..., 2]
    cpos = np.clip(np.searchsorted(uk, ck.ravel()), 0, len(uk) - 1)
    gt_mask = (uk[cpos] == ck.ravel()).reshape(N, M)

    all_N_pred = gt_mask.sum(1).astype(np.float64)
    manifold = (all_N_gt * 2.0) == all_N_pred
    w = manifold.astype(np.float32)

    inv_denom = np.float32(1.0 / max(float(w.sum(dtype=np.float64)) * M, 1.0))
    inv_cnt = np.float32(1.0 / max(float(gm.sum(dtype=np.float64)), 1.0))
    return gt_mask, gm, w, inv_denom, inv_cnt


def _make_shards(x, gt_mask, gm, w, inv_denom, inv_cnt):
    """Build per-core input dicts. x is [N,256] f32."""
    N, M = x.shape
    parts = 128

    # masked-x values padded to L per row (L chosen from data)
    mask_per_row = gt_mask.sum(1)
    L = max(8, int(mask_per_row.max()))
    L = int(2 ** np.ceil(np.log2(L)))
    rr, cc = np.nonzero(gt_mask)
    xm = np.zeros((N, L), np.float32)
    row_starts = np.zeros(N + 1, np.int64)
    np.add.at(row_starts, rr + 1, 1)
    row_starts = np.cumsum(row_starts)
    ranks = np.arange(len(rr)) - row_starts[rr]
    xm[rr, ranks] = x[rr, cc]

    # only manifold rows (w==1) contribute to the main BCE: select them
    sel = np.nonzero(w)[0]
    W = len(sel)
    cap_pc = max(parts, int(np.ceil(W / (N_CORES * parts))) * parts)
    CAP = cap_pc * N_CORES
    xs = np.full((CAP, M), -B_PAD, np.float32)   # pad rows: softplus ~ 1e-13
    xs[:W] = x[sel]
    xms = np.zeros((CAP, L), np.float32)
    xms[:W] = xm[sel]

    # compacted gm groups, padded; distributed evenly over cores
    gn, gi = np.nonzero(gm)               # group ids (row, i)
    total = len(gn)
    per_core = int(np.ceil(total / N_CORES))
    g_chunks = max(1, int(np.ceil(per_core / parts)))  # free-dim group chunks
    cap = g_chunks * parts                       # groups per core
    pl3 = x.reshape(N, 16, 16)

    pad_group = np.full(16, -B_PAD, np.float32)
    pad_group[0] = B_PAD
    pad_group[1] = B_PAD

    in_maps = []
    for core in range(N_CORES):
        s0, s1 = core * cap_pc, (core + 1) * cap_pc
        xc = np.ascontiguousarray(xs[s0:s1].T)          # [256, cap_pc] f32
        kk = cap_pc // parts
        xmc = np.ascontiguousarray(xms[s0:s1]).reshape(parts, kk * L)

        lo, hi = core * per_core, min((core + 1) * per_core, total)
        gsel = np.broadcast_to(pad_group, (cap, 16)).copy()
        if hi > lo:
            gsel[: hi - lo] = pl3[gn[lo:hi], gi[lo:hi], :]
        gsel = np.ascontiguousarray(
            gsel.reshape(g_chunks, parts, 16).transpose(1, 0, 2)
        ).reshape(parts, g_chunks * 16)

        in_maps.append({"x": xc, "xm": xmc, "gsel": gsel})
    return in_maps, L, g_chunks, cap_pc


# ---------------------------------------------------------------- bass build
def _build_bass(L, g_chunks, cap_pc):
    from contextlib import ExitStack

    import concourse.bacc as bacc
    import concourse.mybir as mybir
    import concourse.tile as tile

    f32 = mybir.dt.float32
    bf16 = mybir.dt.bfloat16
    AFT = mybir.ActivationFunctionType
    ALU = mybir.AluOpType
    AX = mybir.AxisListType

    parts, rpp = 128, 16
    G = g_chunks
    S = cap_pc          # selected rows per core
    KK = S // parts     # xm row-chunks per partition

    nc = bacc.Bacc(
        "TRN2", target_bir_lowering=False, debug=False,
        enable_asserts=False, num_devices=N_CORES,
    )
    x_d = nc.dram_tensor("x", [2 * parts, S], f32, kind="ExternalInput").ap()
    xm_d = nc.dram_tensor("xm", [parts, KK * L], f32, kind="ExternalInput").ap()
    g_d = nc.dram_tensor("gsel", [parts, G * 16], f32, kind="ExternalInput").ap()
    out_d = nc.dram_tensor("out", [128, 6], f32, kind="ExternalOutput").ap()

    with tile.TileContext(nc) as tc, ExitStack() as ctx:
        from concourse.tile import add_dep_helper

        def chain(lst):
            for a, b in zip(lst, lst[1:]):
                add_dep_helper(b.ins, a.ins, sync=True, reason="engine order")

        pool = ctx.enter_context(tc.tile_pool(name="main", bufs=1))

        acts = []  # explicit ScalarE program order (avoids table-load thrash)
        dves = []  # pinned DVE order for the post-Max8 tail

        # hoist the exp-table load: dummy activation with no DMA deps
        dumt = pool.tile([1, 8], f32)
        nc.vector.memset(dumt[:], 0.0)
        dumo = pool.tile([1, 8], f32)
        acts.append(nc.scalar.activation(dumo[:], dumt[:], AFT.Exp))

        # --- DMAs: gsel first (feeds the critical Max8 chain) ---
        gt = pool.tile([parts, G * 16], f32)
        NGC = 4
        gch = G * 16 // NGC
        for i in range(NGC):
            nc.sync.dma_start(gt[:, i * gch:(i + 1) * gch],
                              g_d[:, i * gch:(i + 1) * gch])
        halves = []
        for h in range(2):
            xth = pool.tile([parts, S], f32, name=f"xt{h}", tag=f"xt{h}")
            nc.gpsimd.dma_start(xth[:], x_d[h * parts:(h + 1) * parts, :])
            halves.append(xth)
        xmt = pool.tile([parts, KK * L], f32)
        nc.gpsimd.dma_start(xmt[:], xm_d[:])

        # --- exp over gsel (monotonic: Max8 ranks unchanged) interleaved
        #     with the selected-row exps so the ln table switch lands early
        ge = pool.tile([parts, G * 16], f32)
        ets, sps = [], []
        for h in range(2):
            ets.append(pool.tile([parts, S], f32, name=f"e{h}", tag=f"e{h}"))
            sps.append(pool.tile([parts, S], f32, name=f"sp{h}", tag=f"sp{h}"))
        for i in range(2):
            acts.append(nc.scalar.activation(ge[:, i * gch:(i + 1) * gch],
                                             gt[:, i * gch:(i + 1) * gch],
                                             AFT.Exp))
        acts.append(nc.scalar.activation(ets[0][:], halves[0][:], AFT.Exp))
        for i in range(2, NGC):
            acts.append(nc.scalar.activation(ge[:, i * gch:(i + 1) * gch],
                                             gt[:, i * gch:(i + 1) * gch],
                                             AFT.Exp))
        acts.append(nc.scalar.activation(ets[1][:], halves[1][:], AFT.Exp))

        # --- top-8 per compacted gm-group on exp-domain values ---
        top8 = pool.tile([parts, G * 8], f32)
        for g in range(G):
            nc.vector.max(top8[:, g * 8:(g + 1) * 8],
                          ge[:, g * 16:(g + 1) * 16])
        t8e = top8[:].rearrange("p (g e) -> p g e", e=8)
        # pn_cat = [1/exp(pos) , exp(neg)] -> Ln(1+.) gives sp(-pos), sp(neg)
        pn_cat = pool.tile([parts, 2 * G], f32)
        dves.append(nc.vector.reciprocal(pn_cat[:, :G], t8e[:, :, 1]))
        dves.append(nc.vector.tensor_copy(pn_cat[:, G:], t8e[:, :, 2]))

        # --- ln-set phase ---
        for h in range(2):
            acts.append(nc.scalar.activation(sps[h][:], ets[h][:], AFT.Ln,
                                             bias=1.0))
        pn_ln = pool.tile([parts, 2 * G], f32)
        acts.append(nc.scalar.activation(pn_ln[:], pn_cat[:], AFT.Ln, bias=1.0))

        # --- raw partial sums into accs columns; host applies the scales ---
        accs = pool.tile([parts, 6], f32)
        dves.append(nc.vector.tensor_reduce(
            accs[:, 2:3], xmt[:].rearrange("p (k l) -> p k l", l=L),
            axis=AX.XY, op=ALU.add))
        dves.append(nc.vector.tensor_reduce(accs[:, 3:4], sps[0][:],
                                            axis=AX.X, op=ALU.add))
        dves.append(nc.vector.tensor_reduce(accs[:, 4:5], sps[1][:],
                                            axis=AX.X, op=ALU.add))
        dves.append(nc.vector.tensor_reduce(accs[:, 0:1], pn_ln[:, :G],
                                            axis=AX.X, op=ALU.add))
        dves.append(nc.vector.tensor_reduce(accs[:, 1:2], pn_ln[:, G:],
                                            axis=AX.X, op=ALU.add))
        dves.append(nc.vector.memset(accs[:, 5:6], 0.0))
        nc.sync.dma_start(out_d[:], accs[:])

        # pin ScalarE program order: all exp-set work, then all ln-set work
        chain(acts)
        chain(dves)

    if os.environ.get('ATL_PATCH', '0') == '1':
        _prefer_combined_act_table()
    nc.compile()
    return nc


_ACT_PATCHED = False


def _prefer_combined_act_table():
    """Bias bacc's table chooser toward the set holding both Exp and Ln so a
    single ACT_TABLE_LOAD serves the whole kernel."""
    global _ACT_PATCHED
    if _ACT_PATCHED:
        return
    import concourse.bacc as bacc_mod
    import concourse.hw_specs as hw_specs_mod

    orig = hw_specs_mod.get_activation_tables

    def _patched(arch):
        tabs = orig(arch)
        pref = "natural_log_exp_and_others"
        if pref in tabs:
            out = {pref: tabs[pref]}
            out.update({k: v for k, v in tabs.items() if k != pref})
            return out
        return tabs

    bacc_mod.get_activation_tables = _patched
    _ACT_PATCHED = True


# ---------------------------------------------------------------- entrypoint
def _run(pred_logits, points, knn_indices, gt_triangles, **run_kwargs):
    from concourse.bass_utils import run_bass_kernel_spmd

    x = np.ascontiguousarray(np.asarray(pred_logits, dtype=np.float32))
    gt_mask, gm, w, inv_denom, inv_cnt = _host_prep(
        pred_logits, points, knn_indices, gt_triangles)
    in_maps, L, g_chunks, cap_pc = _make_shards(x, gt_mask, gm, w,
                                                 inv_denom, inv_cnt)
    nc = _build_bass(L, g_chunks, cap_pc)
    res = run_bass_kernel_spmd(nc, in_maps, core_ids=list(range(N_CORES)),
                               **run_kwargs)
    acc = np.zeros(6, np.float64)
    for r in res.results:
        acc += np.asarray(r["out"], dtype=np.float64).reshape(128, 6).sum(axis=0)
    pos_t, neg_t, xm_t = acc[0], acc[1], acc[2]
    sp_t = acc[3] + acc[4]
    total = np.array([(sp_t - xm_t) * float(inv_denom),
                      pos_t * float(inv_cnt),
                      neg_t * float(inv_cnt)])
    return total.astype(np.float32), res


def kernel(pred_logits, points, knn_indices, gt_triangles):
    out, _ = _run(pred_logits, points, knn_indices, gt_triangles)
    return out
